# revision 4
# baseline (speedup 1.0000x reference)
"""Balanced CE loss on 8 Trainium2 NeuronCores.

reference: per_el = where(t==1, 2*log(p), 1*log(1-p)); loss = -mean(per_el)

Trick: since the weights are (w0=1, w1=2), select z = t ? p*p : (1-p) and the
per-element value is just log(z) (log(p^2) == 2*log(p)).  One transcendental
pass total.

Per-core engine split (memory-bound target ~94us/core for 32MiB of input):
  DMA  : p tile (f32) + t tile (int32)                ~94us  <- bottleneck
  DVE  : om = 1-p (tensor_scalar, 2x fp32 mode)        ~17us
         z  = copy_predicated(om, t, pp)               ~34us
  ACT  : pp = Square(p); Ln(z) with accum_out          ~55us (same table set)
Partial sums land in a [128, NT] accumulator, DMA'd out; the host sums and
scales (the final reduction over 8*128*NT floats is negligible).
"""

import numpy as np

import concourse.bacc as bacc
import concourse.bass as bass
import concourse.mybir as mybir
import concourse.tile as tile
from concourse.bass_utils import run_bass_kernel_spmd

N = 33554432
NCORES = 8
NSHARD = N // NCORES  # 4194304
P = 128
M = NSHARD // P  # 32768 f32 per partition
F = 2048  # tile free dim -> 1MiB per f32 DMA
NT = M // F  # 16 tiles

WEIGHT0 = 1.0
WEIGHT1 = 2.0

_cache = {}


def build_nc():
    nc = bacc.Bacc("TRN2", target_bir_lowering=False, debug=False, num_devices=NCORES)

    x = nc.dram_tensor("input", [NSHARD], mybir.dt.float32, kind="ExternalInput").ap()
    t = nc.dram_tensor("target", [NSHARD], mybir.dt.int32, kind="ExternalInput").ap()
    out = nc.dram_tensor("out", [P, NT], mybir.dt.float32, kind="ExternalOutput").ap()

    xt = x.rearrange("(n p m) -> n p m", p=P, m=F)
    tt = t.rearrange("(n p m) -> n p m", p=P, m=F)

    with tile.TileContext(nc) as tc:
        with (
            tc.tile_pool(name="io", bufs=4) as io_pool,
            tc.tile_pool(name="work", bufs=4) as work_pool,
            tc.tile_pool(name="acc", bufs=1) as acc_pool,
        ):
            acc = acc_pool.tile([P, NT], mybir.dt.float32)
            for i in range(NT):
                pt = io_pool.tile([P, F], mybir.dt.float32, tag="p")
                nc.sync.dma_start(pt[:], xt[i])
                mt = io_pool.tile([P, F], mybir.dt.int32, tag="t")
                nc.sync.dma_start(mt[:], tt[i])

                # z = 1 - p   (DVE tensor_scalar: (p * -1) + 1, 2x fp32 mode)
                z = work_pool.tile([P, F], mybir.dt.float32, tag="z")
                nc.vector.tensor_scalar(
                    z[:], pt[:], -1.0, 1.0, mybir.AluOpType.mult, mybir.AluOpType.add
                )
                # pp = p^2  (ACT, same table set as Ln)
                pp = work_pool.tile([P, F], mybir.dt.float32, tag="pp")
                nc.scalar.activation(pp[:], pt[:], mybir.ActivationFunctionType.Square)
                # z = t ? pp : z
                nc.vector.copy_predicated(z[:], mt[:], pp[:])
                # ln(z), per-partition sum into acc column i
                lout = work_pool.tile([P, F], mybir.dt.float32, tag="lout")
                nc.scalar.activation(
                    lout[:],
                    z[:],
                    mybir.ActivationFunctionType.Ln,
                    accum_out=acc[:, i : i + 1],
                )

            nc.sync.dma_start(out[:], acc[:])

    nc.compile()
    return nc


def kernel(input, target):
    if "nc" not in _cache:
        _cache["nc"] = build_nc()
    nc = _cache["nc"]

    input = np.ascontiguousarray(np.asarray(input), dtype=np.float32)
    target = np.ascontiguousarray(np.asarray(target), dtype=np.int32)

    in_maps = [
        {
            "input": input[c * NSHARD : (c + 1) * NSHARD],
            "target": target[c * NSHARD : (c + 1) * NSHARD],
        }
        for c in range(NCORES)
    ]
    res = run_bass_kernel_spmd(nc, in_maps, list(range(NCORES)))
    _cache["last_results"] = res

    total = 0.0
    for r in res.results:
        total += r["out"].astype(np.float64).sum()
    return np.asarray(-(total / N), dtype=np.float32)


# revision 8
# speedup vs baseline: 1.2342x; 1.2342x over previous
"""Balanced CE loss on 8 Trainium2 NeuronCores.

reference: per_el = where(t==1, 2*log(p), 1*log(1-p)); loss = -mean(per_el)

Trick: since the weights are (w0=1, w1=2), select z = t ? p*p : (1-p) and the
per-element value is just log(z) (log(p^2) == 2*log(p)).  One transcendental
pass total.

Per-core engine split (memory-bound target ~94us/core for 32MiB of input):
  DMA  : p tile (f32) + t tile (int32)                ~94us  <- bottleneck
  DVE  : om = 1-p (tensor_scalar, 2x fp32 mode)        ~17us
         z  = copy_predicated(om, t, pp)               ~34us
  ACT  : pp = Square(p); Ln(z) with accum_out          ~55us (same table set)
Partial sums land in a [128, NT] accumulator, DMA'd out; the host sums and
scales (the final reduction over 8*128*NT floats is negligible).
"""

import numpy as np

import concourse.bacc as bacc
import concourse.bass as bass
import concourse.mybir as mybir
import concourse.tile as tile
from concourse.bass_utils import run_bass_kernel_spmd

N = 33554432
NCORES = 8
NSHARD = N // NCORES  # 4194304
P = 128
M = NSHARD // P  # 32768 f32 per partition
F = 2048  # bulk tile free dim -> 1MiB per f32 DMA
# Chunk schedule: full-size tiles through the bulk, tapered at the end so the
# post-last-DMA compute chain (SQ -> CP -> LN on the final chunk) is short.
CHUNKS = [1024, 1024] + [2048] * 14 + [512] * 4
assert sum(CHUNKS) == M
NT = len(CHUNKS)

WEIGHT0 = 1.0
WEIGHT1 = 2.0

_cache = {}


def build_nc():
    nc = bacc.Bacc("TRN2", target_bir_lowering=False, debug=False, num_devices=NCORES)

    x = nc.dram_tensor("input", [NSHARD], mybir.dt.float32, kind="ExternalInput").ap()
    t = nc.dram_tensor("target", [NSHARD], mybir.dt.int32, kind="ExternalInput").ap()
    out = nc.dram_tensor("out", [P, NT], mybir.dt.float32, kind="ExternalOutput").ap()

    xt = x.rearrange("(p m) -> p m", p=P)
    tt = t.rearrange("(p m) -> p m", p=P)

    with tile.TileContext(nc) as tc:
        with (
            tc.tile_pool(name="io", bufs=6) as io_pool,
            tc.tile_pool(name="zp", bufs=3) as z_pool,
            tc.tile_pool(name="pq", bufs=3) as pp_pool,
            tc.tile_pool(name="ln", bufs=2, space="PSUM") as ln_pool,
            tc.tile_pool(name="acc", bufs=1) as acc_pool,
        ):
            acc = acc_pool.tile([P, NT], mybir.dt.float32)
            off = 0
            for i, w in enumerate(CHUNKS):
                pt = io_pool.tile([P, w], mybir.dt.float32, tag="p")
                nc.sync.dma_start(pt[:], xt[:, off : off + w])
                mt = io_pool.tile([P, w], mybir.dt.int32, tag="t")
                nc.sync.dma_start(mt[:], tt[:, off : off + w])

                # z = 1 - p   (DVE tensor_scalar: (p * -1) + 1, 2x fp32 mode)
                z = z_pool.tile([P, w], mybir.dt.float32, tag="z")
                nc.vector.tensor_scalar(
                    z[:], pt[:], -1.0, 1.0, mybir.AluOpType.mult, mybir.AluOpType.add
                )
                # pp = p^2  (ACT, same table set as Ln)
                pp = pp_pool.tile([P, w], mybir.dt.float32, tag="pp")
                nc.scalar.activation(pp[:], pt[:], mybir.ActivationFunctionType.Square)
                # z = t ? pp : z
                nc.vector.copy_predicated(z[:], mt[:], pp[:])
                # ln(z), per-partition sum into acc column i; the full-size Ln
                # output is dead -> dump it in PSUM to save SBUF
                lout = ln_pool.tile([P, w], mybir.dt.float32, tag="lout")
                nc.scalar.activation(
                    lout[:],
                    z[:],
                    mybir.ActivationFunctionType.Ln,
                    accum_out=acc[:, i : i + 1],
                )
                off += w

            nc.sync.dma_start(out[:], acc[:])

    nc.compile()
    return nc


def kernel(input, target):
    if "nc" not in _cache:
        _cache["nc"] = build_nc()
    nc = _cache["nc"]

    input = np.ascontiguousarray(np.asarray(input), dtype=np.float32)
    target = np.ascontiguousarray(np.asarray(target), dtype=np.int32)

    in_maps = [
        {
            "input": input[c * NSHARD : (c + 1) * NSHARD],
            "target": target[c * NSHARD : (c + 1) * NSHARD],
        }
        for c in range(NCORES)
    ]
    res = run_bass_kernel_spmd(nc, in_maps, list(range(NCORES)))
    _cache["last_results"] = res

    total = 0.0
    for r in res.results:
        total += r["out"].astype(np.float64).sum()
    return np.asarray(-(total / N), dtype=np.float32)


# revision 11
# speedup vs baseline: 1.2441x; 1.0081x over previous
"""Balanced CE loss on 8 Trainium2 NeuronCores — raw Bass (hand-synced).

Math: z = t ? p*p : (1-p); loss = -mean(ln z)   (ln(p^2) == 2 ln p, w1=2, w0=1)

Engine split per chunk i (width w):
  Sync  : dma p_i -> +16 pl[i%NL] ; dma t_i -> +16 tl[i%NL]
  GpSimd: OM_i  z = 1 - p           -> +1 s_om
  ACT   : SQ_i  pp = p^2            -> +1 s_sq
          LN_{i-1} ln(z), accum col -> +1 s_ln     (one-stage pipelined)
  DVE   : CP_i  z = t ? pp : z      -> +1 s_cp

DMA completion uses round-robin lane sems (a single sem cannot prove a given
tile landed when several DMAs are in flight: their 16 per-engine increments
interleave).  NRT does not reset semaphores between invocations, so ours are
cleared at the END of the kernel (past the Block-exit barrier); each run then
starts from zero with no start-of-kernel fence, letting the first DMA issue
as soon as the Sync engine boots.  The tail chunks taper so the
post-last-DMA compute chain is short.
"""

import numpy as np

import concourse.bacc as bacc
import concourse.bass as bass
import concourse.mybir as mybir
from concourse.bass_utils import run_bass_kernel_spmd

N = 33554432
NCORES = 8
NSHARD = N // NCORES  # 4194304
P = 128
M = NSHARD // P  # 32768 f32 per partition

F = 2048  # slot width (max chunk width)
CHUNKS = [2048] * 15 + [512] * 4
assert sum(CHUNKS) == M
NT = len(CHUNKS)

KP = 7  # p-tile slots
KT = 7  # t-tile slots
KZ = 4  # z slots
KPP = 4  # pp slots
NL = 4  # DMA completion lane sems per stream

WEIGHT0 = 1.0
WEIGHT1 = 2.0

_cache = {}

AF = mybir.ActivationFunctionType
ALU = mybir.AluOpType


def build_nc():
    # Bass.__init__ ends with an all_engine_barrier after the const memsets.
    # That barrier makes every engine wait for the slowest-booting one (~3.4us
    # measured, dominated by the unused PE/Tensor engine) before any DMA can
    # issue. Skip it; the only ordering it provided that this kernel needs is
    # const-memsets (GpSimd) vs ACT's bias read, covered by the s_const
    # handshake below.
    _orig_barrier = bass.Bass.all_engine_barrier
    bass.Bass.all_engine_barrier = lambda self, *a, **k: None
    try:
        nc = bacc.Bacc(
            "TRN2", target_bir_lowering=False, debug=False, num_devices=NCORES
        )
    finally:
        bass.Bass.all_engine_barrier = _orig_barrier

    x = nc.dram_tensor("input", [NSHARD], mybir.dt.float32, kind="ExternalInput").ap()
    t = nc.dram_tensor("target", [NSHARD], mybir.dt.int32, kind="ExternalInput").ap()
    out = nc.dram_tensor("out", [P, NT], mybir.dt.float32, kind="ExternalOutput").ap()

    xt = x.rearrange("(p m) -> p m", p=P)
    tt = t.rearrange("(p m) -> p m", p=P)

    offs = []
    o = 0
    for w in CHUNKS:
        offs.append(o)
        o += w

    pl = [nc.alloc_semaphore(f"s_p{j}") for j in range(NL)]
    tl = [nc.alloc_semaphore(f"s_t{j}") for j in range(NL)]
    s_out = nc.alloc_semaphore("s_out")
    s_sq = nc.alloc_semaphore("s_sq")
    s_om = nc.alloc_semaphore("s_om")
    s_cp = nc.alloc_semaphore("s_cp")
    s_ln = nc.alloc_semaphore("s_ln")
    s_const = nc.alloc_semaphore("s_const")
    sems = pl + tl + [s_out, s_sq, s_om, s_cp, s_ln, s_const]

    def p_done(eng, i):  # wait until p chunk i fully landed
        eng.wait_ge(pl[i % NL], 16 * (i // NL + 1))

    def t_done(eng, i):
        eng.wait_ge(tl[i % NL], 16 * (i // NL + 1))

    # Sems start at 0: zeroed by NRT at model load, and re-zeroed by OUR
    # end-of-kernel clears (after the Block-exit barrier) on every run. So no
    # start-of-kernel fence is needed and the first DMA can issue as soon as
    # the Sync engine boots. The only start-time ordering needed is the
    # framework's const memsets (GpSimd) vs ACT's bias read: a one-sem
    # handshake below covers it.

    with (
        nc.sbuf_tensor([P, KP * F], mybir.dt.float32) as pbuf,
        nc.sbuf_tensor([P, KT * F], mybir.dt.int32) as tbuf,
        nc.sbuf_tensor([P, KZ * F], mybir.dt.float32) as zbuf,
        nc.sbuf_tensor([P, KPP * F], mybir.dt.float32) as ppbuf,
        nc.sbuf_tensor([P, NT], mybir.dt.float32) as acc,
        nc.sbuf_tensor([P, 1], mybir.dt.float32) as dummy,
        nc.psum_tensor([P, F], mybir.dt.float32) as ln0,
        nc.psum_tensor([P, F], mybir.dt.float32) as ln1,
        nc.Block() as block,
    ):
        lnouts = [ln0, ln1]

        # GpSimd ran the framework's const memsets in its preamble; publish
        # their completion for ACT (which reads the const-0.0 bias AP).
        @block.gpsimd
        def _(gp):
            gp.memset(dummy[:, :], 0.0).then_inc(s_const)

        def pslot(i, w):
            return pbuf[:, (i % KP) * F : (i % KP) * F + w]

        def tslot(i, w):
            return tbuf[:, (i % KT) * F : (i % KT) * F + w]

        def zslot(i, w):
            return zbuf[:, (i % KZ) * F : (i % KZ) * F + w]

        def ppslot(i, w):
            return ppbuf[:, (i % KPP) * F : (i % KPP) * F + w]

        # ---- Sync: DMA issue, paced by slot-free sems --------------------
        @block.sync
        def _(sync):
            for i, w in enumerate(CHUNKS):
                if i >= KP:
                    sync.wait_ge(s_sq, i - KP + 1)
                    sync.wait_ge(s_om, i - KP + 1)
                if i >= NL:
                    sync.wait_ge(pl[i % NL], 16 * (i // NL))
                sync.dma_start(
                    out=pslot(i, w), in_=xt[:, offs[i] : offs[i] + w]
                ).then_inc(pl[i % NL], 16)
                if i >= KT:
                    sync.wait_ge(s_cp, i - KT + 1)
                if i >= NL:
                    sync.wait_ge(tl[i % NL], 16 * (i // NL))
                sync.dma_start(
                    out=tslot(i, w), in_=tt[:, offs[i] : offs[i] + w]
                ).then_inc(tl[i % NL], 16)
            sync.wait_ge(s_ln, NT)
            sync.dma_start(out=out[:], in_=acc[:]).then_inc(s_out, 16)
            sync.wait_ge(s_out, 16)

        # ---- ACT: pp = p^2 ; ln(z) with accum, one-stage pipelined ------
        @block.scalar
        def _(scalar):
            scalar.wait_ge(s_const, 1)
            def sq(i):
                p_done(scalar, i)
                if i >= KPP:
                    scalar.wait_ge(s_cp, i - KPP + 1)
                scalar.activation(
                    ppslot(i, CHUNKS[i]), pslot(i, CHUNKS[i]), AF.Square
                ).then_inc(s_sq)

            def ln(i):
                w = CHUNKS[i]
                scalar.wait_ge(s_cp, i + 1)
                if i >= 2:
                    # orders the PSUM dump-slot WAW with LN_{i-2} (always
                    # already satisfied; needed for the race detector)
                    scalar.wait_ge(s_ln, i - 1)
                scalar.activation(
                    lnouts[i % 2][:, :w],
                    zslot(i, w),
                    AF.Ln,
                    accum_out=acc[:, i : i + 1],
                ).then_inc(s_ln)

            sq(0)
            for i in range(1, NT):
                sq(i)
                ln(i - 1)
            ln(NT - 1)

        # ---- DVE: z = 1 - p ; z = t ? pp : z ----------------------------
        # (NOT on GpSimd: it shares the SBUF port with DVE under an
        # exclusive lock, so GpSimd work serializes against copy_predicated)
        @block.vector
        def _(vector):
            for i, w in enumerate(CHUNKS):
                if i >= KZ:
                    vector.wait_ge(s_ln, i - KZ + 1)
                p_done(vector, i)
                vector.tensor_scalar(
                    zslot(i, w), pslot(i, w), -1.0, 1.0, ALU.mult, ALU.add
                ).then_inc(s_om)
                t_done(vector, i)
                vector.wait_ge(s_sq, i + 1)
                # same-engine WAW with the tensor_scalar above through the
                # DVE pipeline
                vector.wait_ge(s_om, i + 1)
                vector.copy_predicated(zslot(i, w), tslot(i, w), ppslot(i, w)).then_inc(
                    s_cp
                )

    # Past the Block-exit barrier every engine is done: reset our sems (and
    # the DMA state tied to them) so the next invocation starts from zero.
    for r in bass.compact_to_ranges([s.num for s in sems]):
        nc.gpsimd.dma_reset(r)
        nc.gpsimd.sem_clear(r)

    nc.compile()
    return nc


def kernel(input, target):
    if "nc" not in _cache:
        _cache["nc"] = build_nc()
    nc = _cache["nc"]

    input = np.ascontiguousarray(np.asarray(input), dtype=np.float32)
    target = np.ascontiguousarray(np.asarray(target), dtype=np.int32)

    in_maps = [
        {
            "input": input[c * NSHARD : (c + 1) * NSHARD],
            "target": target[c * NSHARD : (c + 1) * NSHARD],
        }
        for c in range(NCORES)
    ]
    res = run_bass_kernel_spmd(nc, in_maps, list(range(NCORES)))
    _cache["last_results"] = res

    total = 0.0
    for r in res.results:
        total += r["out"].astype(np.float64).sum()
    return np.asarray(-(total / N), dtype=np.float32)
